# revision 2
# baseline (speedup 1.0000x reference)
"""Causal self-attention (B=4, T=2048, C=1024, H=16, D=64) on 8 TRN2 NeuronCores.

Sharding: core c handles batch b=c//2 and head-group g=c%2 (8 of 16 heads,
i.e. 512 of 1024 feature columns). Each core projects q,k,v for its heads,
runs causal softmax(q k^T / sqrt(d)) v, and computes the partial o_proj
attn_out[:, cols] @ Wo[:, cols].T -> [T, C]. Host sums the two head-group
partials per batch and stacks batches.

Kernel structure (streamed over 8 tq-chunks of 256):
  - scores are computed transposed per head pair: S^T[tk, tq] (K=64, M=128 tk)
    from bf16 kT/qT; exp on the Activation engine writes bf16 probabilities
    pt[tk, 2, tq]; diagonal 128-blocks are masked in place by a gpsimd
    affine_select.
  - PV runs in the NATURAL orientation: out[tq, d] with lhsT = pt (K=tk,
    M=128 tq fully used), rhs = v[tk, d] (N=64) - half the PE cost of the
    transposed form. Softmax denominators accumulate in parallel via N=1
    matmuls against a ones-column. Normalization is a single per-partition
    broadcast multiply (denominator lives on the tq partition - no DRAM
    broadcast round-trip needed).
  - the normalized attn [tq, f] is flipped with PE transposes (f32r identity,
    1.5 cycles/row) into attnT [f, tq] for the o_proj matmuls.
  - k/v/q projections for chunk c+1 and o_proj for chunk c-1 are interleaved
    into chunk c's exp-paced inner loop so the PE never starves.
"""

import numpy as np

B, T, C, H, D = 4, 2048, 1024, 16, 64
NCORES = 8
FH = 512          # features per core = 8 heads
NCT = C // 128    # 8 contraction tiles
JP = 4            # head-pair tiles (8 heads / 2)
CH = 256          # tq chunk width
NCH = T // CH     # 8 chunks

_CACHE = {}


def _build():
    import concourse.bass as bass  # noqa: F401
    import concourse.mybir as mybir
    from concourse import bacc
    from concourse.tile import TileContext

    F32 = mybir.dt.float32
    F32R = mybir.dt.float32r
    BF16 = mybir.dt.bfloat16
    EXP = mybir.ActivationFunctionType.Exp

    nc = bacc.Bacc("TRN2", target_bir_lowering=False, debug=False, num_devices=NCORES)
    xT_h = nc.dram_tensor("xT", [C, T], F32R, kind="ExternalInput")
    wq_h = nc.dram_tensor("wqT", [C, FH], F32R, kind="ExternalInput")
    wk_h = nc.dram_tensor("wkT", [C, FH], F32R, kind="ExternalInput")
    wv_h = nc.dram_tensor("wvT", [C, FH], F32R, kind="ExternalInput")
    wo_h = nc.dram_tensor("woT", [FH, C], F32R, kind="ExternalInput")
    out_h = nc.dram_tensor("out", [T, C], F32, kind="ExternalOutput")
    xT = xT_h.ap()
    out_ap = out_h.ap()

    with TileContext(nc) as tc:
        with (
            tc.tile_pool(name="persist", bufs=1) as persist,
            tc.tile_pool(name="xp", bufs=2) as xp,
            tc.tile_pool(name="qp", bufs=2) as qp,
            tc.tile_pool(name="ptp", bufs=4) as ptp,
            tc.tile_pool(name="asbp", bufs=2) as asbp,
            tc.tile_pool(name="atp", bufs=2) as atp,
            tc.tile_pool(name="opl", bufs=3) as opool,
            tc.tile_pool(name="rp", bufs=2) as rp,
            tc.tile_pool(name="pvp", bufs=1, space="PSUM") as pvp,
            tc.tile_pool(name="sp", bufs=2, space="PSUM") as sp,
            tc.tile_pool(name="pp", bufs=2, space="PSUM") as pp,
            tc.tile_pool(name="dnp", bufs=1, space="PSUM") as dnp,
            tc.tile_pool(name="tpp", bufs=1, space="PSUM") as tpp,
        ):
            wq_s = persist.tile([128, NCT, FH], F32R, tag="wq")
            wk_s = persist.tile([128, NCT, FH], F32R, tag="wk")
            wv_s = persist.tile([128, NCT, FH], F32R, tag="wv")
            wo_s = persist.tile([128, JP, C], F32R, tag="wo")
            kT_s = persist.tile([128, JP, T], BF16, tag="kT")
            v_s = persist.tile([128, 2 * NCH, 8, D], BF16, tag="vs")

            wk_src = wk_h.ap().rearrange("(c p) f -> p c f", p=128)
            wv_src = wv_h.ap().rearrange("(c p) f -> p c f", p=128)

            def load_xt(c):
                t = xp.tile([128, NCT, CH], F32R, tag="xt")
                src = xT[:, c * CH:(c + 1) * CH].rearrange("(c p) t -> p c t", p=128)
                nc.sync.dma_start(out=t[:, 0:4, :], in_=src[:, 0:4, :])
                nc.sync.dma_start(out=t[:, 4:8, :], in_=src[:, 4:8, :])
                return t

            # startup DMAs: wk + xt0 first (k-proj is the first PE work)
            xt0 = xp.tile([128, NCT, CH], F32R, tag="xt")
            xt0_src = xT[:, 0:CH].rearrange("(c p) t -> p c t", p=128)
            for cc in range(0, NCT, 2):
                nc.sync.dma_start(out=wk_s[:, cc:cc + 2, :], in_=wk_src[:, cc:cc + 2, :])
                nc.sync.dma_start(out=xt0[:, cc // 2 * 2:cc // 2 * 2 + 2, :],
                                  in_=xt0_src[:, cc:cc + 2, :])
            nc.sync.dma_start(out=wv_s[:, 0:4, :], in_=wv_src[:, 0:4, :])
            nc.sync.dma_start(out=wv_s[:, 4:8, :], in_=wv_src[:, 4:8, :])
            nc.sync.dma_start(out=wq_s, in_=wq_h.ap().rearrange("(c p) f -> p c f", p=128))
            nc.sync.dma_start(out=wo_s, in_=wo_h.ap().rearrange("(i p) f -> p i f", p=128))

            # ones column for the denominator matmuls
            ones = persist.tile([128, 1], BF16, tag="ones")
            nc.gpsimd.memset(ones, 1.0)
            # f32r identity for PE transposes
            idn = persist.tile([128, 128], F32R, tag="idn")
            nc.gpsimd.memset(idn, 1.0)
            nc.gpsimd.affine_select(
                out=idn, in_=idn, compare_op=mybir.AluOpType.is_ge, fill=0.0,
                base=0, pattern=[[1, 128]], channel_multiplier=-1,
            )
            nc.gpsimd.affine_select(
                out=idn, in_=idn, compare_op=mybir.AluOpType.is_le, fill=0.0,
                base=0, pattern=[[1, 128]], channel_multiplier=-1,
            )

            def kv_steps(c, xt_t):
                # k^T and v projections for chunk c (writes kT_s window + v_s)
                for j in range(JP):
                    ps = pp.tile([128, 512], F32, tag="pp")
                    for cc in range(NCT):
                        nc.tensor.matmul(
                            ps[:, 0:CH], wk_s[:, cc, j * 128:(j + 1) * 128],
                            xt_t[:, cc, :],
                            start=(cc == 0), stop=(cc == NCT - 1), skip_group_check=True,
                        )
                        yield
                    nc.vector.tensor_copy(out=kT_s[:, j, c * CH:(c + 1) * CH],
                                          in_=ps[:, 0:CH])
                    yield
                for tt in range(2):
                    ps = pp.tile([128, 512], F32, tag="pp")
                    for cc in range(NCT):
                        nc.tensor.matmul(
                            ps, xt_t[:, cc, tt * 128:(tt + 1) * 128], wv_s[:, cc, :],
                            start=(cc == 0), stop=(cc == NCT - 1), skip_group_check=True,
                        )
                        yield
                    nc.vector.tensor_copy(
                        out=v_s[:, 2 * c + tt, :, :],
                        in_=ps.rearrange("p (h d) -> p h d", h=8),
                    )
                    yield

            def q_steps(c, xt_t, qT_t):
                for j in range(JP):
                    ps = pp.tile([128, 512], F32, tag="pp")
                    for cc in range(NCT):
                        nc.tensor.matmul(
                            ps[:, 0:CH], wq_s[:, cc, j * 128:(j + 1) * 128],
                            xt_t[:, cc, :],
                            start=(cc == 0), stop=(cc == NCT - 1), skip_group_check=True,
                        )
                        yield
                    nc.vector.tensor_copy(out=qT_t[:, j, :], in_=ps[:, 0:CH])
                    yield

            def o_steps(c, at):
                for n in range(2):
                    for mt in range(2):
                        po = pp.tile([128, 512], F32, tag="pp")
                        for i in range(JP):
                            nc.tensor.matmul(
                                po, at[:, i, mt * 128:(mt + 1) * 128],
                                wo_s[:, i, n * 512:(n + 1) * 512],
                                start=(i == 0), stop=(i == JP - 1), skip_group_check=True,
                            )
                            yield
                        ot = opool.tile([128, 512], F32, tag="ot")
                        nc.vector.tensor_copy(out=ot, in_=po)
                        nc.sync.dma_start(
                            out=out_ap[c * CH + mt * 128: c * CH + (mt + 1) * 128,
                                       n * 512:(n + 1) * 512],
                            in_=ot,
                        )
                        yield

            def chain(*gens):
                for g in gens:
                    yield from g

            SENT = object()

            # ---- prologue: kv + q projections for chunk 0 (nothing to
            # overlap with yet) ----
            qT_cur = qp.tile([128, JP, CH], BF16, tag="qT")
            for _ in kv_steps(0, xt0):
                pass
            for _ in q_steps(0, xt0, qT_cur):
                pass
            xt_next = load_xt(1)  # consumed by chunk-1 projections during chunk 0

            attnT_prev = None
            for c in range(NCH):
                nkt = 2 * c + 2
                if c + 2 < NCH:
                    xt_next2 = load_xt(c + 2)
                gens = []
                n_items = 0
                if attnT_prev is not None:
                    gens.append(o_steps(c - 1, attnT_prev))
                    n_items += 20
                if c + 1 < NCH:
                    qT_next = qp.tile([128, JP, CH], BF16, tag="qT")
                    gens.append(kv_steps(c + 1, xt_next))
                    gens.append(q_steps(c + 1, xt_next, qT_next))
                    n_items += 74
                stream = chain(*gens)
                lead = 4 if c == 0 else 2
                S_c = JP * nkt
                emitted = 0
                idx = 0

                pv = pvp.tile([128, 2, 8, D], F32, tag="pv")
                den = dnp.tile([128, 2, 256], F32, tag="den")

                def emit_pv(j, kt, pt):
                    for hh in range(2):
                        h = 2 * j + hh
                        for a in range(2):
                            ig = 2 * c + a
                            if kt <= ig:
                                nc.tensor.matmul(
                                    pv[:, a, h, :], pt[:, hh, a * 128:(a + 1) * 128],
                                    v_s[:, kt, h, :],
                                    start=(kt == 0), stop=(kt == ig),
                                    skip_group_check=True,
                                )
                                nc.tensor.matmul(
                                    den[:, a, h:h + 1], pt[:, hh, a * 128:(a + 1) * 128],
                                    ones,
                                    start=(kt == 0), stop=(kt == ig),
                                    skip_group_check=True,
                                )

                pend = None
                for j in range(JP):
                    for kt in range(nkt):
                        c0 = 128 if kt == 2 * c + 1 else 0
                        s = sp.tile([128, 2, CH], F32, tag="s")
                        nc.tensor.matmul(
                            s[:, 0, c0:CH], kT_s[0:64, j, kt * 128:(kt + 1) * 128],
                            qT_cur[0:64, j, c0:CH], start=True, stop=True,
                        )
                        nc.tensor.matmul(
                            s[:, 1, c0:CH], kT_s[64:128, j, kt * 128:(kt + 1) * 128],
                            qT_cur[64:128, j, c0:CH], start=True, stop=True,
                        )
                        pt = ptp.tile([128, 2, CH], BF16, tag="pt")
                        if c0 == 0:
                            nc.scalar.activation(out=pt, in_=s, func=EXP, scale=0.125)
                        else:
                            nc.scalar.activation(out=pt[:, :, c0:CH], in_=s[:, :, c0:CH],
                                                 func=EXP, scale=0.125)
                        if kt >= 2 * c:
                            # zero the upper triangle of the diagonal 128-block
                            nc.gpsimd.affine_select(
                                out=pt[:, :, c0:c0 + 128], in_=pt[:, :, c0:c0 + 128],
                                compare_op=mybir.AluOpType.is_ge, fill=0.0,
                                base=0, pattern=[[0, 2], [1, 128]],
                                channel_multiplier=-1,
                            )
                        idx += 1
                        want = n_items * max(0, idx - lead) // max(1, S_c - lead)
                        while emitted < want:
                            if next(stream, SENT) is SENT:
                                emitted = n_items
                                break
                            emitted += 1
                        if pend is not None:
                            emit_pv(*pend)
                        pend = (j, kt, pt)
                emit_pv(*pend)
                for _ in stream:
                    pass

                # chunk epilogue: normalize, transpose to [f, tq]
                rec = rp.tile([128, 2, 8], F32, tag="rec")
                nc.vector.reciprocal(out=rec, in_=den[:, :, 0:8])
                asb = asbp.tile([128, 2, 8, D], F32R, tag="asb")
                for a in range(2):
                    nc.vector.tensor_mul(
                        asb[:, a], pv[:, a],
                        rec[:, a, :, None].broadcast_to([128, 8, D]),
                    )
                asb2 = asb.rearrange("p a h d -> p a (h d)")
                attnT_cur = atp.tile([128, JP, CH], F32R, tag="attnT")
                for half in range(2):
                    tp = tpp.tile([128, 2, 256], F32R, tag="tp")
                    for k in range(2):
                        fb = 2 * half + k
                        for a in range(2):
                            nc.tensor.transpose(
                                tp[:, k, a * 128:(a + 1) * 128],
                                asb2[:, a, fb * 128:(fb + 1) * 128], idn,
                            )
                    nc.vector.tensor_copy(
                        out=attnT_cur[:, 2 * half:2 * half + 2, :], in_=tp)
                attnT_prev = attnT_cur
                if c + 1 < NCH:
                    qT_cur = qT_next
                    if c + 2 < NCH:
                        xt_next = xt_next2

            # tail o_proj for the last chunk
            for _ in o_steps(NCH - 1, attnT_prev):
                pass

    nc.compile()
    return nc


def _get_nc():
    if "nc" not in _CACHE:
        _CACHE["nc"] = _build()
    return _CACHE["nc"]


def make_in_maps(x, Wq, Wk, Wv, Wo):
    x = np.asarray(x, dtype=np.float32)
    Wq = np.asarray(Wq, dtype=np.float32)
    Wk = np.asarray(Wk, dtype=np.float32)
    Wv = np.asarray(Wv, dtype=np.float32)
    Wo = np.asarray(Wo, dtype=np.float32)
    in_maps = []
    for core in range(NCORES):
        b, g = core // 2, core % 2
        cols = slice(FH * g, FH * (g + 1))
        in_maps.append({
            "xT": np.ascontiguousarray(x[b].T),
            "wqT": np.ascontiguousarray(Wq.T[:, cols]),
            "wkT": np.ascontiguousarray(Wk.T[:, cols]),
            "wvT": np.ascontiguousarray(Wv.T[:, cols]),
            "woT": np.ascontiguousarray(Wo.T[cols, :]),
        })
    return in_maps


def gather_out(parts):
    return np.stack([parts[2 * b] + parts[2 * b + 1] for b in range(B)])


def kernel(x, Wq, Wk, Wv, Wo):
    from concourse.bass_utils import run_bass_kernel_spmd

    nc = _get_nc()
    in_maps = make_in_maps(x, Wq, Wk, Wv, Wo)
    try:
        res = run_bass_kernel_spmd(nc, in_maps, core_ids=list(range(NCORES)))
    except Exception:
        # transient NRT device errors have been observed on this fabric;
        # one retry costs nothing when healthy
        res = run_bass_kernel_spmd(nc, in_maps, core_ids=list(range(NCORES)))
    return gather_out([res.results[c]["out"] for c in range(NCORES)])
